# revision 55
# baseline (speedup 1.0000x reference)
"""CustomGaussianLayer Trainium2 kernel (bf16, packed-DMA).

Math: out[b,o] = sum_{i,g} exp(-a*(tanh(x[b,i])-c_g)^2) * coeff[o,i,g]*W[o,i]
 == E @ W2T  with  E[b, k=(i,g)] Gaussian basis,  W2T[k, o] folded weights,
 a = 24.5, centers c_g = linspace(-1,1,8).

Factored basis:  exp(-a(t-c)^2) = A * B_g * exp(-a c^2)
  A   = exp(-a t^2)                 (ACT, bf16 out)
  B_g = exp(2 a c_g t)              even g: ACT exp (bf16); odd: B_{g-1}*r (DVE)
  r   = exp(2 a dc t) = exp(14 t)   (ACT, bf16),  exp(-a c^2) folded into W2T

Per core (data-parallel over batch, 1024 rows each): E/W2T bf16, PE fp32
into 8 psum banks [4 o-tiles x 2 b-chunks], out bf16 upcast on host.
Inputs are host-packed into the exact SBUF image so every DMA runs with
2-16KB contiguous descriptors on the Sync hardware queue.  PE warm-up
matmuls (on uninitialized SBUF, no memset gate) defeat the HAM cold clock.
The first i-block runs bc-major with 256-col basis slivers so real
matmuls start ~11-12us; the last two g-blocks retire bank-major so
drains + output DMA overlap the final matmuls.
"""

import numpy as np
import ml_dtypes

import concourse.bacc as bacc
import concourse.bass as bass
import concourse.mybir as mybir
import concourse.tile as tile
from concourse.bass_utils import run_bass_kernel_spmd
from concourse.tile import add_dep_helper

G = 8
I_SZ = 512
O_SZ = 512
B = 8192
NCORES = 8
B_SH = B // NCORES          # 1024 batch rows per core
K = I_SZ * G                # 4096 contraction
N_IBLK = I_SZ // 128        # 4 partition blocks of i
FREE = N_IBLK * B_SH        # 4096 free layout (i_blk, b)
N_OT = O_SZ // 128          # 4 output tiles
N_BC = B_SH // 512          # 2 batch chunks of 512 (psum free limit fp32)
N_KT = K // 128             # 32 k tiles

ALPHA = 24.5
N_WARMUP = 11                # dummy matmuls to beat the HAM cold clock
CENTERS = np.linspace(-1.0, 1.0, G).astype(np.float64)
DC = float(CENTERS[1] - CENTERS[0])          # 2/7
R_SCALE = float(2.0 * ALPHA * DC)            # 14.0

F32 = mybir.dt.float32
BF16 = mybir.dt.bfloat16
AF = mybir.ActivationFunctionType
ALU = mybir.AluOpType

_NC_CACHE = {}


def build_nc():
    nc = bacc.Bacc("TRN2", target_bir_lowering=False)
    # all tensors pre-packed on host into the SBUF image layout
    xt_d = nc.dram_tensor("xt", [128, FREE], F32, kind="ExternalInput")
    w2t_d = nc.dram_tensor("w2t", [128, N_KT * O_SZ], BF16, kind="ExternalInput")
    out_d = nc.dram_tensor("out_t", [128, N_OT * B_SH], BF16,
                           kind="ExternalOutput")

    with tile.TileContext(nc) as tc:
        with (
            tc.tile_pool(name="scr", bufs=1) as scr_pool,
            tc.tile_pool(name="w2", bufs=1) as w2_pool,
            tc.tile_pool(name="xt", bufs=1) as xt_pool,
            tc.tile_pool(name="tt", bufs=1) as tt_pool,
            tc.tile_pool(name="sq", bufs=2) as sq_pool,
            tc.tile_pool(name="rr", bufs=3) as rr_pool,
            tc.tile_pool(name="aa", bufs=3) as aa_pool,
            tc.tile_pool(name="bb", bufs=4) as bb_pool,
            tc.tile_pool(name="ee", bufs=8) as ee_pool,
            tc.tile_pool(name="ps", bufs=1, space="PSUM") as ps_pool,
            tc.tile_pool(name="ob", bufs=1) as ob_pool,
        ):
            # ---- warm-up scaffolding, all DMA-independent ----
            w2_all = w2_pool.tile([128, N_KT * O_SZ], BF16, tag="w2all")
            xt_sb = xt_pool.tile([128, FREE], F32, tag="xt")

            # ACT spline-table preload (exp/tanh share one table set).
            # Input is the uninitialized tail of w2_all (DMA'd ~25us later;
            # WAR dep is harmless) so no memset gates the table load.
            actwarm = scr_pool.tile([128, 1], F32, tag="actwarm")
            actwarm_i = nc.scalar.activation(
                actwarm[:], w2_all[:, 16383:16384], AF.Exp)

            psum = [
                [
                    ps_pool.tile(
                        [128, 512], F32,
                        name=f"ps{ot}_{bc}", tag=f"ps{ot}_{bc}",
                    )
                    for bc in range(N_BC)
                ]
                for ot in range(N_OT)
            ]
            # warm-ups on the uninitialized tail of w2_all: no memset gate,
            # so the PE starts the HAM busy window right out of preamble
            for w in range(N_WARMUP):
                nc.tensor.matmul(
                    psum[0][0][:], w2_all[:, 15744:15872],
                    w2_all[:, 15872:16384],
                    start=(w == 0), stop=(w == N_WARMUP - 1),
                )

            # ---- input DMA: one Sync hw queue, strict deadline order.
            # kt0-3 leads (it gates the first real matmuls anyway and this
            # keeps the early 256KB chunks coarse), then xt/w2 interleaved.
            dma_chain = [
                nc.sync.dma_start(xt_sb[:, 0:256], xt_d[:, 0:256]),
                nc.sync.dma_start(w2_all[:, 0:1024], w2t_d[:, 0:1024]),
                nc.sync.dma_start(w2_all[:, 1024:2048], w2t_d[:, 1024:2048]),
                nc.sync.dma_start(w2_all[:, 2048:4096], w2t_d[:, 2048:4096]),
                nc.sync.dma_start(xt_sb[:, 512:1024], xt_d[:, 512:1024]),
                nc.sync.dma_start(xt_sb[:, 256:512], xt_d[:, 256:512]),
                nc.sync.dma_start(w2_all[:, 4096:8192], w2t_d[:, 4096:8192]),
                nc.sync.dma_start(xt_sb[:, 1024:2048], xt_d[:, 1024:2048]),
                nc.sync.dma_start(xt_sb[:, 2048:4096], xt_d[:, 2048:4096]),
                nc.sync.dma_start(w2_all[:, 8192:16384], w2t_d[:, 8192:16384]),
            ]
            for i in range(1, len(dma_chain)):
                add_dep_helper(dma_chain[i].ins, dma_chain[i - 1].ins,
                               sync=False, reason="DMA lane consumer order")

            # ---- engine program-order chains ----
            act_chain = [actwarm_i]
            dve_chain = []

            def act(out_ap, in_ap, fn, scale=1.0, name=None):
                i = nc.scalar.activation(out_ap, in_ap, fn, scale=float(scale))
                add_dep_helper(i.ins, act_chain[-1].ins, sync=False,
                               reason="ACT program order")
                act_chain.append(i)
                return i

            def dve(out_ap, in0, in1):
                i = nc.vector.tensor_tensor(out_ap, in0, in1, op=ALU.mult)
                if dve_chain:
                    add_dep_helper(i.ins, dve_chain[-1].ins, sync=False,
                                   reason="DVE program order")
                dve_chain.append(i)
                return i

            # ---- basis helper: cols [lo,hi) of tt -> e_tiles[g][lo-off:hi-off]
            def basis(lo, hi, e_tiles, off, label, even_first=False):
                w = hi - lo
                sl = slice(lo, hi)
                sq_t = sq_pool.tile([128, w], F32, tag="sq", name=f"sq_{label}")
                a_t = aa_pool.tile([128, w], BF16, tag="aa", name=f"a_{label}")
                r_t = rr_pool.tile([128, w], BF16, tag="rr", name=f"r_{label}")
                b_e = {}
                act(tt[:, sl], xt_sb[:, sl], AF.Tanh)
                g_order = ([0, 2, 4, 6, 1, 3, 5, 7] if even_first
                           else list(range(G)))
                for g in g_order:
                    c = float(CENTERS[g])
                    if g % 2 == 0:
                        b_t = bb_pool.tile([128, w], BF16, tag="bb",
                                           name=f"b_{label}_{g}")
                        act(b_t[:], tt[:, sl], AF.Exp, scale=2.0 * ALPHA * c)
                        if g == 0:
                            dve(sq_t[:], tt[:, sl], tt[:, sl])
                            act(a_t[:], sq_t[:], AF.Exp, scale=-ALPHA)
                            act(r_t[:], tt[:, sl], AF.Exp, scale=R_SCALE)
                    else:
                        b_t = bb_pool.tile([128, w], BF16, tag="bbo",
                                           name=f"b_{label}_{g}")
                        dve(b_t[:], b_e[g - 1][:], r_t[:])
                    b_e[g] = b_t
                    dve(e_tiles[g][:, lo - off:hi - off], a_t[:], b_t[:])

            # ---- matmul emission with per-bank start tracking ----
            started = set()

            def mmg(kt, bc, e_ap, off, wdt):
                """4 matmuls (all ot) into cols [off,off+wdt) of bank col bc.

                start=True only on a bank's very first matmul: it clears the
                whole bank's has_written bits, so later no-start matmuls on
                other column ranges overwrite-on-first-touch correctly."""
                for ot in range(N_OT):
                    lhsT = w2_all[:, kt * O_SZ + ot * 128:
                                  kt * O_SZ + (ot + 1) * 128]
                    first = (ot, bc) not in started
                    started.add((ot, bc))
                    nc.tensor.matmul(
                        psum[ot][bc][:, off:off + wdt], lhsT, e_ap,
                        start=first, stop=False, skip_group_check=True,
                    )

            def mm4(kt, bc, e_ap512):
                mmg(kt, bc, e_ap512, 0, 512)

            tt = tt_pool.tile([128, FREE], F32, tag="tt")
            e_h0 = [ee_pool.tile([128, 2048], BF16, tag="ee", name=f"e_h0_{g}")
                    for g in range(G)]

            # h0-ib0: 256-col startup sliver, then bc1 full, then the second
            # bc0 sliver (its slow 256-col basis hides under the 512 pass)
            basis(0, 256, e_h0, 0, "s0")
            for g in [0, 2, 1, 4, 3, 6, 5, 7]:
                mmg(g, 0, e_h0[g][:, 0:256], 0, 256)
            basis(512, 1024, e_h0, 0, "s2")
            for g in range(G):
                mm4(g, 1, e_h0[g][:, 512:1024])
            basis(256, 512, e_h0, 0, "s1")
            for g in range(G):
                mmg(g, 0, e_h0[g][:, 256:512], 256, 256)
            basis(1024, 2048, e_h0, 0, "s3")
            # PE pass: ib1 both bc
            for g in range(G):
                for bc in range(N_BC):
                    mm4(8 + g, bc, e_h0[g][:, 1024 + bc * 512:1536 + bc * 512])

            # h1: full-width basis per g, (g, ib) interleaved PE
            sqh = sq_pool.tile([128, 2048], F32, tag="sqh")
            ah = aa_pool.tile([128, 2048], BF16, tag="aah")
            rh = rr_pool.tile([128, 2048], BF16, tag="rrh")
            sl = slice(2048, 4096)
            act(tt[:, sl], xt_sb[:, sl], AF.Tanh)
            b_prev = None
            e_h1 = {}
            for g in range(G):
                c = float(CENTERS[g])
                e_t = ee_pool.tile([128, 2048], BF16, tag="ee",
                                   name=f"e_h1_{g}")
                e_h1[g] = e_t
                if g % 2 == 0:
                    b_t = bb_pool.tile([128, 2048], BF16, tag="bbh",
                                       name=f"bh_{g}")
                    act(b_t[:], tt[:, sl], AF.Exp, scale=2.0 * ALPHA * c)
                    if g == 0:
                        dve(sqh[:], tt[:, sl], tt[:, sl])
                        act(ah[:], sqh[:], AF.Exp, scale=-ALPHA)
                        act(rh[:], tt[:, sl], AF.Exp, scale=R_SCALE)
                else:
                    b_t = bb_pool.tile([128, 2048], BF16, tag="bbh",
                                       name=f"bh_{g}")
                    dve(b_t[:], b_prev[:], rh[:])
                b_prev = b_t
                dve(e_t[:], ah[:], b_t[:])
                if g < G - 2:
                    for ib_l in range(2):
                        for bc in range(N_BC):
                            mm4(16 + g * 2 + ib_l, bc,
                                e_t[:, ib_l * 1024 + bc * 512:
                                    ib_l * 1024 + bc * 512 + 512])

            # last two g-blocks bank-major: each bank retires early so its
            # drain + output DMA overlap the remaining matmuls
            o_sb = ob_pool.tile([128, N_OT * B_SH], BF16, tag="osb")
            for ot in range(N_OT):
                for bc in range(N_BC):
                    for g in (G - 2, G - 1):
                        for ib_l in range(2):
                            kt = 16 + g * 2 + ib_l
                            lhsT = w2_all[:, kt * O_SZ + ot * 128:
                                          kt * O_SZ + (ot + 1) * 128]
                            nc.tensor.matmul(
                                psum[ot][bc][:], lhsT,
                                e_h1[g][:, ib_l * 1024 + bc * 512:
                                        ib_l * 1024 + bc * 512 + 512],
                                start=False,
                                stop=(g == G - 1 and ib_l == 1),
                                skip_group_check=True,
                            )
                    dst = o_sb[:, (ot * N_BC + bc) * 512:
                               (ot * N_BC + bc + 1) * 512]
                    last_bank = (ot == N_OT - 1 and bc == N_BC - 1)
                    if last_bank:
                        # split the kernel's critical-path drain across both
                        # engines so it halves
                        ci = nc.vector.tensor_copy(
                            dst[:, 0:256], psum[ot][bc][:, 0:256])
                        add_dep_helper(ci.ins, dve_chain[-1].ins, sync=False,
                                       reason="DVE program order")
                        dve_chain.append(ci)
                        di = nc.scalar.activation(
                            dst[:, 256:512], psum[ot][bc][:, 256:512], AF.Copy)
                        add_dep_helper(di.ins, act_chain[-1].ins, sync=False,
                                       reason="ACT program order")
                        act_chain.append(di)
                    elif bc == 0:
                        ci = nc.vector.tensor_copy(dst, psum[ot][bc][:])
                        add_dep_helper(ci.ins, dve_chain[-1].ins, sync=False,
                                       reason="DVE program order")
                        dve_chain.append(ci)
                    else:
                        di = nc.scalar.activation(dst, psum[ot][bc][:], AF.Copy)
                        add_dep_helper(di.ins, act_chain[-1].ins, sync=False,
                                       reason="ACT program order")
                        act_chain.append(di)
                if ot < N_OT - 1:
                    out_eng = nc.sync if ot % 2 == 0 else nc.scalar
                    out_eng.dma_start(
                        out_d[:, ot * B_SH:(ot + 1) * B_SH],
                        o_sb[:, ot * B_SH:(ot + 1) * B_SH],
                    )
                else:
                    # last o-tile: per-bc halves, the final one on the idle
                    # Sync queue right after the split drain
                    nc.scalar.dma_start(
                        out_d[:, ot * B_SH:ot * B_SH + 512],
                        o_sb[:, ot * B_SH:ot * B_SH + 512],
                    )
                    nc.sync.dma_start(
                        out_d[:, ot * B_SH + 512:(ot + 1) * B_SH],
                        o_sb[:, ot * B_SH + 512:(ot + 1) * B_SH],
                    )
    nc.compile()
    return nc


def get_nc():
    if "nc" not in _NC_CACHE:
        _NC_CACHE["nc"] = build_nc()
    return _NC_CACHE["nc"]


def prep_inputs(x, weights, coefficients):
    x = np.asarray(x, dtype=np.float32)
    weights = np.asarray(weights, dtype=np.float32)
    coefficients = np.asarray(coefficients, dtype=np.float32)
    # W2T[k=g*I+i, o] = coeff[o,i,g] * W[o,i] * exp(-a c_g^2)
    w2t = (coefficients * weights[:, :, None]).transpose(2, 1, 0).reshape(K, O_SZ)
    gauss_bias = np.exp(-ALPHA * CENTERS ** 2)  # [G]
    w2t = (w2t.reshape(G, I_SZ, O_SZ) * gauss_bias[:, None, None]).astype(np.float32)
    # k-tile order: h0: kt = ib*8 + g (ib 0,1); h1: kt = 16 + g*2 + (ib-2)
    w2t = w2t.reshape(G, N_IBLK, 128, O_SZ)            # [g, ib, p, o]
    tiles = np.empty((N_KT, 128, O_SZ), dtype=np.float32)
    for ib in range(2):
        for g in range(G):
            tiles[ib * 8 + g] = w2t[g, ib]
    for g in range(G):
        for ib in range(2):
            tiles[16 + g * 2 + ib] = w2t[g, 2 + ib]
    # pack to SBUF image [128p, kt*O]
    w2_img = np.ascontiguousarray(
        tiles.transpose(1, 0, 2).reshape(128, N_KT * O_SZ)
    ).astype(ml_dtypes.bfloat16)
    xT = np.ascontiguousarray(x.T)  # [I, B]
    in_maps = []
    for c in range(NCORES):
        xc = xT[:, c * B_SH:(c + 1) * B_SH]            # [512, 1024]
        xt_img = np.ascontiguousarray(
            xc.reshape(N_IBLK, 128, B_SH).transpose(1, 0, 2).reshape(128, FREE)
        )
        in_maps.append({"xt": xt_img, "w2t": w2_img})
    return in_maps


def unpack_out(res):
    out = np.empty((B, O_SZ), dtype=np.float32)
    for c in range(NCORES):
        o_img = np.asarray(res.results[c]["out_t"]).astype(np.float32)
        # [128, ot*1024+b] -> [O, B_SH] -> [B_SH, O]
        o_full = o_img.reshape(128, N_OT, B_SH).transpose(1, 0, 2) \
                      .reshape(O_SZ, B_SH)
        out[c * B_SH:(c + 1) * B_SH, :] = o_full.T
    return out


def kernel(x, weights, coefficients):
    nc = get_nc()
    in_maps = prep_inputs(x, weights, coefficients)
    res = run_bass_kernel_spmd(nc, in_maps, core_ids=list(range(NCORES)))
    return unpack_out(res)


# revision 56
# speedup vs baseline: 1.1766x; 1.1766x over previous
"""CustomGaussianLayer Trainium2 kernel (bf16, packed-DMA).

Math: out[b,o] = sum_{i,g} exp(-a*(tanh(x[b,i])-c_g)^2) * coeff[o,i,g]*W[o,i]
 == E @ W2T  with  E[b, k=(i,g)] Gaussian basis,  W2T[k, o] folded weights,
 a = 24.5, centers c_g = linspace(-1,1,8).

Factored basis:  exp(-a(t-c)^2) = A * B_g * exp(-a c^2)
  A   = exp(-a t^2)                 (ACT, bf16 out)
  B_g = exp(2 a c_g t)              even g: ACT exp (bf16); odd: B_{g-1}*r (DVE)
  r   = exp(2 a dc t) = exp(14 t)   (ACT, bf16),  exp(-a c^2) folded into W2T

Per core (data-parallel over batch, 1024 rows each): E/W2T bf16, PE fp32
into 8 psum banks [4 o-tiles x 2 b-chunks], out bf16 upcast on host.
Inputs are host-packed into the exact SBUF image so every DMA runs with
2-16KB contiguous descriptors on the Sync hardware queue.  PE warm-up
matmuls (on uninitialized SBUF, no memset gate) defeat the HAM cold clock.
The first i-block runs bc-major with 256-col basis slivers so real
matmuls start ~11-12us; the last two g-blocks retire bank-major so
drains + output DMA overlap the final matmuls.
"""

import numpy as np
import ml_dtypes

import concourse.bacc as bacc
import concourse.bass as bass
import concourse.mybir as mybir
import concourse.tile as tile
from concourse.bass_utils import run_bass_kernel_spmd
from concourse.tile import add_dep_helper

G = 8
I_SZ = 512
O_SZ = 512
B = 8192
NCORES = 8
B_SH = B // NCORES          # 1024 batch rows per core
K = I_SZ * G                # 4096 contraction
N_IBLK = I_SZ // 128        # 4 partition blocks of i
FREE = N_IBLK * B_SH        # 4096 free layout (i_blk, b)
N_OT = O_SZ // 128          # 4 output tiles
N_BC = B_SH // 512          # 2 batch chunks of 512 (psum free limit fp32)
N_KT = K // 128             # 32 k tiles

ALPHA = 24.5
N_WARMUP = 10                # dummy matmuls to beat the HAM cold clock
CENTERS = np.linspace(-1.0, 1.0, G).astype(np.float64)
DC = float(CENTERS[1] - CENTERS[0])          # 2/7
R_SCALE = float(2.0 * ALPHA * DC)            # 14.0

F32 = mybir.dt.float32
BF16 = mybir.dt.bfloat16
AF = mybir.ActivationFunctionType
ALU = mybir.AluOpType

_NC_CACHE = {}


def build_nc():
    nc = bacc.Bacc("TRN2", target_bir_lowering=False)
    # all tensors pre-packed on host into the SBUF image layout
    xt_d = nc.dram_tensor("xt", [128, FREE], F32, kind="ExternalInput")
    w2t_d = nc.dram_tensor("w2t", [128, N_KT * O_SZ], BF16, kind="ExternalInput")
    out_d = nc.dram_tensor("out_t", [128, N_OT * B_SH], BF16,
                           kind="ExternalOutput")

    with tile.TileContext(nc) as tc:
        with (
            tc.tile_pool(name="scr", bufs=1) as scr_pool,
            tc.tile_pool(name="w2", bufs=1) as w2_pool,
            tc.tile_pool(name="xt", bufs=1) as xt_pool,
            tc.tile_pool(name="tt", bufs=1) as tt_pool,
            tc.tile_pool(name="sq", bufs=2) as sq_pool,
            tc.tile_pool(name="rr", bufs=3) as rr_pool,
            tc.tile_pool(name="aa", bufs=3) as aa_pool,
            tc.tile_pool(name="bb", bufs=4) as bb_pool,
            tc.tile_pool(name="ee", bufs=8) as ee_pool,
            tc.tile_pool(name="ps", bufs=1, space="PSUM") as ps_pool,
            tc.tile_pool(name="ob", bufs=1) as ob_pool,
        ):
            # ---- warm-up scaffolding, all DMA-independent ----
            w2_all = w2_pool.tile([128, N_KT * O_SZ], BF16, tag="w2all")
            xt_sb = xt_pool.tile([128, FREE], F32, tag="xt")

            # ACT spline-table preload (exp/tanh share one table set).
            # Input is the uninitialized tail of w2_all (DMA'd ~25us later;
            # WAR dep is harmless) so no memset gates the table load.
            actwarm = scr_pool.tile([128, 1], F32, tag="actwarm")
            actwarm_i = nc.scalar.activation(
                actwarm[:], w2_all[:, 16383:16384], AF.Exp)

            psum = [
                [
                    ps_pool.tile(
                        [128, 512], F32,
                        name=f"ps{ot}_{bc}", tag=f"ps{ot}_{bc}",
                    )
                    for bc in range(N_BC)
                ]
                for ot in range(N_OT)
            ]
            # warm-ups on the uninitialized tail of w2_all: no memset gate,
            # so the PE starts the HAM busy window right out of preamble
            for w in range(N_WARMUP):
                nc.tensor.matmul(
                    psum[0][0][:], w2_all[:, 15744:15872],
                    w2_all[:, 15872:16384],
                    start=(w == 0), stop=(w == N_WARMUP - 1),
                )

            # ---- input DMA: one Sync hw queue, strict deadline order.
            # kt0-3 leads (it gates the first real matmuls anyway and this
            # keeps the early 256KB chunks coarse), then xt/w2 interleaved.
            dma_chain = [
                nc.sync.dma_start(xt_sb[:, 0:256], xt_d[:, 0:256]),
                nc.sync.dma_start(w2_all[:, 0:1024], w2t_d[:, 0:1024]),
                nc.sync.dma_start(w2_all[:, 1024:2048], w2t_d[:, 1024:2048]),
                nc.sync.dma_start(w2_all[:, 2048:4096], w2t_d[:, 2048:4096]),
                nc.sync.dma_start(xt_sb[:, 512:1024], xt_d[:, 512:1024]),
                nc.sync.dma_start(xt_sb[:, 256:512], xt_d[:, 256:512]),
                nc.sync.dma_start(w2_all[:, 4096:8192], w2t_d[:, 4096:8192]),
                nc.sync.dma_start(xt_sb[:, 1024:2048], xt_d[:, 1024:2048]),
                nc.sync.dma_start(xt_sb[:, 2048:4096], xt_d[:, 2048:4096]),
                nc.sync.dma_start(w2_all[:, 8192:16384], w2t_d[:, 8192:16384]),
            ]
            for i in range(1, len(dma_chain)):
                add_dep_helper(dma_chain[i].ins, dma_chain[i - 1].ins,
                               sync=False, reason="DMA lane consumer order")

            # ---- engine program-order chains ----
            act_chain = [actwarm_i]
            dve_chain = []

            def act(out_ap, in_ap, fn, scale=1.0, name=None):
                i = nc.scalar.activation(out_ap, in_ap, fn, scale=float(scale))
                add_dep_helper(i.ins, act_chain[-1].ins, sync=False,
                               reason="ACT program order")
                act_chain.append(i)
                return i

            def dve(out_ap, in0, in1):
                i = nc.vector.tensor_tensor(out_ap, in0, in1, op=ALU.mult)
                if dve_chain:
                    add_dep_helper(i.ins, dve_chain[-1].ins, sync=False,
                                   reason="DVE program order")
                dve_chain.append(i)
                return i

            # ---- basis helper: cols [lo,hi) of tt -> e_tiles[g][lo-off:hi-off]
            def basis(lo, hi, e_tiles, off, label, even_first=False):
                w = hi - lo
                sl = slice(lo, hi)
                sq_t = sq_pool.tile([128, w], F32, tag="sq", name=f"sq_{label}")
                a_t = aa_pool.tile([128, w], BF16, tag="aa", name=f"a_{label}")
                r_t = rr_pool.tile([128, w], BF16, tag="rr", name=f"r_{label}")
                b_e = {}
                act(tt[:, sl], xt_sb[:, sl], AF.Tanh)
                g_order = ([0, 2, 4, 6, 1, 3, 5, 7] if even_first
                           else list(range(G)))
                for g in g_order:
                    c = float(CENTERS[g])
                    if g % 2 == 0:
                        b_t = bb_pool.tile([128, w], BF16, tag="bb",
                                           name=f"b_{label}_{g}")
                        act(b_t[:], tt[:, sl], AF.Exp, scale=2.0 * ALPHA * c)
                        if g == 0:
                            dve(sq_t[:], tt[:, sl], tt[:, sl])
                            act(a_t[:], sq_t[:], AF.Exp, scale=-ALPHA)
                            act(r_t[:], tt[:, sl], AF.Exp, scale=R_SCALE)
                    else:
                        b_t = bb_pool.tile([128, w], BF16, tag="bbo",
                                           name=f"b_{label}_{g}")
                        dve(b_t[:], b_e[g - 1][:], r_t[:])
                    b_e[g] = b_t
                    dve(e_tiles[g][:, lo - off:hi - off], a_t[:], b_t[:])

            # ---- matmul emission with per-bank start tracking ----
            started = set()

            def mmg(kt, bc, e_ap, off, wdt):
                """4 matmuls (all ot) into cols [off,off+wdt) of bank col bc.

                start=True only on a bank's very first matmul: it clears the
                whole bank's has_written bits, so later no-start matmuls on
                other column ranges overwrite-on-first-touch correctly."""
                for ot in range(N_OT):
                    lhsT = w2_all[:, kt * O_SZ + ot * 128:
                                  kt * O_SZ + (ot + 1) * 128]
                    first = (ot, bc) not in started
                    started.add((ot, bc))
                    nc.tensor.matmul(
                        psum[ot][bc][:, off:off + wdt], lhsT, e_ap,
                        start=first, stop=False, skip_group_check=True,
                    )

            def mm4(kt, bc, e_ap512):
                mmg(kt, bc, e_ap512, 0, 512)

            tt = tt_pool.tile([128, FREE], F32, tag="tt")
            e_h0 = [ee_pool.tile([128, 2048], BF16, tag="ee", name=f"e_h0_{g}")
                    for g in range(G)]

            # h0-ib0: 256-col startup sliver, then bc1 full, then the second
            # bc0 sliver (its slow 256-col basis hides under the 512 pass)
            basis(0, 256, e_h0, 0, "s0")
            for g in [0, 2, 1, 4, 3, 6, 5, 7]:
                mmg(g, 0, e_h0[g][:, 0:256], 0, 256)
            basis(512, 1024, e_h0, 0, "s2")
            for g in range(G):
                mm4(g, 1, e_h0[g][:, 512:1024])
            basis(256, 512, e_h0, 0, "s1")
            for g in range(G):
                mmg(g, 0, e_h0[g][:, 256:512], 256, 256)
            basis(1024, 2048, e_h0, 0, "s3")
            # PE pass: ib1 both bc
            for g in range(G):
                for bc in range(N_BC):
                    mm4(8 + g, bc, e_h0[g][:, 1024 + bc * 512:1536 + bc * 512])

            # h1: full-width basis per g, (g, ib) interleaved PE
            sqh = sq_pool.tile([128, 2048], F32, tag="sqh")
            ah = aa_pool.tile([128, 2048], BF16, tag="aah")
            rh = rr_pool.tile([128, 2048], BF16, tag="rrh")
            sl = slice(2048, 4096)
            act(tt[:, sl], xt_sb[:, sl], AF.Tanh)
            b_prev = None
            e_h1 = {}
            for g in range(G):
                c = float(CENTERS[g])
                e_t = ee_pool.tile([128, 2048], BF16, tag="ee",
                                   name=f"e_h1_{g}")
                e_h1[g] = e_t
                if g % 2 == 0:
                    b_t = bb_pool.tile([128, 2048], BF16, tag="bbh",
                                       name=f"bh_{g}")
                    act(b_t[:], tt[:, sl], AF.Exp, scale=2.0 * ALPHA * c)
                    if g == 0:
                        dve(sqh[:], tt[:, sl], tt[:, sl])
                        act(ah[:], sqh[:], AF.Exp, scale=-ALPHA)
                        act(rh[:], tt[:, sl], AF.Exp, scale=R_SCALE)
                else:
                    b_t = bb_pool.tile([128, 2048], BF16, tag="bbh",
                                       name=f"bh_{g}")
                    dve(b_t[:], b_prev[:], rh[:])
                b_prev = b_t
                dve(e_t[:], ah[:], b_t[:])
                if g < G - 2:
                    for ib_l in range(2):
                        for bc in range(N_BC):
                            mm4(16 + g * 2 + ib_l, bc,
                                e_t[:, ib_l * 1024 + bc * 512:
                                    ib_l * 1024 + bc * 512 + 512])

            # last two g-blocks bank-major: each bank retires early so its
            # drain + output DMA overlap the remaining matmuls
            o_sb = ob_pool.tile([128, N_OT * B_SH], BF16, tag="osb")
            for ot in range(N_OT):
                for bc in range(N_BC):
                    for g in (G - 2, G - 1):
                        for ib_l in range(2):
                            kt = 16 + g * 2 + ib_l
                            lhsT = w2_all[:, kt * O_SZ + ot * 128:
                                          kt * O_SZ + (ot + 1) * 128]
                            nc.tensor.matmul(
                                psum[ot][bc][:], lhsT,
                                e_h1[g][:, ib_l * 1024 + bc * 512:
                                        ib_l * 1024 + bc * 512 + 512],
                                start=False,
                                stop=(g == G - 1 and ib_l == 1),
                                skip_group_check=True,
                            )
                    dst = o_sb[:, (ot * N_BC + bc) * 512:
                               (ot * N_BC + bc + 1) * 512]
                    last_bank = (ot == N_OT - 1 and bc == N_BC - 1)
                    if last_bank:
                        # split the kernel's critical-path drain across both
                        # engines so it halves
                        ci = nc.vector.tensor_copy(
                            dst[:, 0:256], psum[ot][bc][:, 0:256])
                        add_dep_helper(ci.ins, dve_chain[-1].ins, sync=False,
                                       reason="DVE program order")
                        dve_chain.append(ci)
                        di = nc.scalar.activation(
                            dst[:, 256:512], psum[ot][bc][:, 256:512], AF.Copy)
                        add_dep_helper(di.ins, act_chain[-1].ins, sync=False,
                                       reason="ACT program order")
                        act_chain.append(di)
                    elif bc == 0:
                        ci = nc.vector.tensor_copy(dst, psum[ot][bc][:])
                        add_dep_helper(ci.ins, dve_chain[-1].ins, sync=False,
                                       reason="DVE program order")
                        dve_chain.append(ci)
                    else:
                        di = nc.scalar.activation(dst, psum[ot][bc][:], AF.Copy)
                        add_dep_helper(di.ins, act_chain[-1].ins, sync=False,
                                       reason="ACT program order")
                        act_chain.append(di)
                if ot < N_OT - 1:
                    out_eng = nc.sync if ot % 2 == 0 else nc.scalar
                    out_eng.dma_start(
                        out_d[:, ot * B_SH:(ot + 1) * B_SH],
                        o_sb[:, ot * B_SH:(ot + 1) * B_SH],
                    )
                else:
                    # last o-tile: per-bc halves, the final one on the idle
                    # Sync queue right after the split drain
                    nc.scalar.dma_start(
                        out_d[:, ot * B_SH:ot * B_SH + 512],
                        o_sb[:, ot * B_SH:ot * B_SH + 512],
                    )
                    nc.sync.dma_start(
                        out_d[:, ot * B_SH + 512:(ot + 1) * B_SH],
                        o_sb[:, ot * B_SH + 512:(ot + 1) * B_SH],
                    )
    nc.compile()
    return nc


def get_nc():
    if "nc" not in _NC_CACHE:
        _NC_CACHE["nc"] = build_nc()
    return _NC_CACHE["nc"]


def prep_inputs(x, weights, coefficients):
    x = np.asarray(x, dtype=np.float32)
    weights = np.asarray(weights, dtype=np.float32)
    coefficients = np.asarray(coefficients, dtype=np.float32)
    # W2T[k=g*I+i, o] = coeff[o,i,g] * W[o,i] * exp(-a c_g^2)
    w2t = (coefficients * weights[:, :, None]).transpose(2, 1, 0).reshape(K, O_SZ)
    gauss_bias = np.exp(-ALPHA * CENTERS ** 2)  # [G]
    w2t = (w2t.reshape(G, I_SZ, O_SZ) * gauss_bias[:, None, None]).astype(np.float32)
    # k-tile order: h0: kt = ib*8 + g (ib 0,1); h1: kt = 16 + g*2 + (ib-2)
    w2t = w2t.reshape(G, N_IBLK, 128, O_SZ)            # [g, ib, p, o]
    tiles = np.empty((N_KT, 128, O_SZ), dtype=np.float32)
    for ib in range(2):
        for g in range(G):
            tiles[ib * 8 + g] = w2t[g, ib]
    for g in range(G):
        for ib in range(2):
            tiles[16 + g * 2 + ib] = w2t[g, 2 + ib]
    # pack to SBUF image [128p, kt*O]
    w2_img = np.ascontiguousarray(
        tiles.transpose(1, 0, 2).reshape(128, N_KT * O_SZ)
    ).astype(ml_dtypes.bfloat16)
    xT = np.ascontiguousarray(x.T)  # [I, B]
    in_maps = []
    for c in range(NCORES):
        xc = xT[:, c * B_SH:(c + 1) * B_SH]            # [512, 1024]
        xt_img = np.ascontiguousarray(
            xc.reshape(N_IBLK, 128, B_SH).transpose(1, 0, 2).reshape(128, FREE)
        )
        in_maps.append({"xt": xt_img, "w2t": w2_img})
    return in_maps


def unpack_out(res):
    out = np.empty((B, O_SZ), dtype=np.float32)
    for c in range(NCORES):
        o_img = np.asarray(res.results[c]["out_t"]).astype(np.float32)
        # [128, ot*1024+b] -> [O, B_SH] -> [B_SH, O]
        o_full = o_img.reshape(128, N_OT, B_SH).transpose(1, 0, 2) \
                      .reshape(O_SZ, B_SH)
        out[c * B_SH:(c + 1) * B_SH, :] = o_full.T
    return out


def kernel(x, weights, coefficients):
    nc = get_nc()
    in_maps = prep_inputs(x, weights, coefficients)
    res = run_bass_kernel_spmd(nc, in_maps, core_ids=list(range(NCORES)))
    return unpack_out(res)
